# revision 41
# baseline (speedup 1.0000x reference)
"""MoE FeedForward (top-2 of 8 experts, SwiGLU) for 8 Trainium2 NeuronCores.

H-sharded layout (v4): instead of expert-parallel (one expert per core,
capacity = max expert load = 1092), every core processes ALL routed
(token, expert) pairs but only a 512-wide slice of the hidden dim H
(h-tiles 4i..4i+3 for core i).  The MoE combine (host) sums the 8
partial outputs and applies the gates.  Per-core PE work is exactly
sum(L_e)/8-equivalent = 1024 token-slots -- perfect balance, ~6% less
matmul streaming than the expert-parallel v3.

The host router (fp32 top-2 + softmax) runs before build_program, so
the per-expert token counts C_e are compile-time constants: the program
loops over experts with exact trip counts.  All cores run the SAME
program (SPMD); they differ only in which h-slice of W1/W2/W3 their
input tensors contain.

Per-core phases, per expert e (in process order):
  B(e): hh[ht, tok] = silu(W1_e[:, hsl].T @ x_e) * (W2_e[:, hsl].T @ x_e)
        for the core's 4 h-tiles; ht outer, token-chunks inner; x_e
        resident (2.2 MB), weights streamed once (524 KB/h-tile).
  C(e): out_partial[d, tok] = W3_e[hsl, :].T @ hh  -- accumulation
        depth only 4 k-tiles; dt outer / kh inner so each PSUM bank
        finishes fast and evictions overlap the next bank's matmuls.
C(e) is emitted after B(e); B(e+1) follows, its DMAs prefetched during
B(e), so the PE stream never waits on memory in steady state.

All matmuls fp16 (PE full rate; fp8 would need DoubleRow but its
~2.6% quantization rms blows the 2e-2 rel-err budget -- measured).
"""

import contextlib

import numpy as np

import concourse.bacc as bacc
import concourse.tile as tile
from concourse import mybir
from concourse.bass import ds, ts
from concourse.bass_utils import run_bass_kernel_spmd

AF = mybir.ActivationFunctionType
F32 = mybir.dt.float32
F16 = mybir.dt.float16

# Problem shape (hardcoded per contract)
B, S, D, H, E = 2, 2048, 1024, 4096, 8
N = B * S            # 4096 tokens
TOP_K = 2
NCORES = 8

P = 128              # SBUF partitions
KD = D // P          # 8 k-tiles over D
HTL = (H // P) // NCORES  # 4 h-tiles (of 128) per core
DT = D // P          # 8 d-tiles (phase C stationary tiles)
WARMUP = 24          # dummy matmuls to hold the HAM clock at 8/8
                     # through the ~8.5us startup head


def round8(v):
    return (v + 7) & ~7


def round4(v):
    return (v + 3) & ~3


def make_plans(loads):
    """Per-expert (C_e, chunk list) in process order, plus the order.

    Experts are processed largest-load first (the last expert is the
    smallest so the kernel tail is short).  The first expert leads with
    progressively growing chunks (128/256/360) so early matmuls depend
    on small DMAs; the last expert trails with a 64-token chunk so the
    final eviction+store tail after the last matmul is tiny.
    """
    order = sorted(range(E), key=lambda e: -loads[e])
    plans = []
    for i, e in enumerate(order):
        C_e = round4(max(loads[e], 8))
        rem = C_e
        lead, tail = [], []
        if i == 0 and C_e >= 752:
            lead = [128, 256, 360]
            rem -= 744
        if i == E - 1 and rem >= 72:
            tail = [64]
            rem -= 64
        n = max(1, -(-rem // 512))
        base = round8(-(-rem // n))
        mids = []
        while rem > 0:
            c = min(base, rem)
            mids.append(c)
            rem -= c
        assert all(c % 4 == 0 and 0 < c <= 512 for c in lead + mids + tail)
        plans.append((C_e, lead + mids + tail))
    return order, plans


def build_program(plans):
    nc = bacc.Bacc(
        "TRN2",
        target_bir_lowering=False,
        debug=False,
        enable_asserts=False,
        num_devices=NCORES,
    )
    C_list = [int(p[0]) for p in plans]
    C_tot = sum(C_list)
    C_max = max(C_list)
    offs = [int(v) for v in np.cumsum([0] + C_list)[:-1]]

    # Host-pre-shuffled layouts (see make_in_maps):
    #   x_d [p, KD*off_e + KD*c0 + k*cw + t] = x_routed_e[c0+t, k*128+p]
    #     (chunk-major within each expert block)
    #   w12_d[p, e, ht, j*KD*128 + k*128 + h] = Wj_e[k*128+p, (4i+ht)*128+h]
    #   w3_d [p, e, kh*D + d]                 = W3_e[(4i+kh)*128+p, d]
    #   out_d[D, C_tot] f16 partials over this core's h-slice (fp16
#     store halves the output traffic; the host combine sums in f32 and
#     the rounding is ~1e-5 of the 2e-2 budget)
    x_d = nc.dram_tensor("xc", [P, KD * C_tot], F16, kind="ExternalInput").ap()
    w12_d = nc.dram_tensor(
        "W12", [P, E, HTL, 2 * KD * P], F16, kind="ExternalInput"
    ).ap()
    w3_d = nc.dram_tensor("W3e", [P, E, HTL * D], F16, kind="ExternalInput").ap()
    out_d = nc.dram_tensor("out", [D, C_tot], F16, kind="ExternalOutput").ap()
    out_v = out_d.rearrange("(dt p) c -> p dt c", p=P)    # [128, DT, C_tot]

    with tile.TileContext(nc) as tc:
        with contextlib.ExitStack() as ctx:
            singles = ctx.enter_context(tc.tile_pool(name="singles", bufs=1))
            w12p = ctx.enter_context(tc.tile_pool(name="w12", bufs=6))
            xrp = ctx.enter_context(tc.tile_pool(name="xr", bufs=2))
            hhp = ctx.enter_context(tc.tile_pool(name="hh", bufs=2))
            w3p = ctx.enter_context(tc.tile_pool(name="w3", bufs=2))
            evp = ctx.enter_context(tc.tile_pool(name="ev", bufs=3))
            obp = ctx.enter_context(tc.tile_pool(name="ob", bufs=6))
            psp = ctx.enter_context(tc.tile_pool(name="ps", bufs=8, space="PSUM"))

            # --- startup DMAs -------------------------------------------------
            # Early per-queue DMA bandwidth is low (~100-200 GB/s while
            # the queues ramp from ~8.5us), so the first-matmul
            # dependencies are spread over the three DMA rings and split
            # into small pieces, each its own tile (tile-granular
            # dependency tracking would otherwise make the first matmul
            # wait for all of them):
            #   sync:   w12(e0,ht0) as W1|W2 halves, then w12(e0,ht1..3)
            #   scalar: all x0 chunks (later: w12 streaming); gpsimd
            #           starts ~2us later and slower, it keeps only the
            #           output stores
            # Expert 0's phase B runs a triangular (ht, chunk) schedule
            # that consumes tiles in roughly this landing order.
            w12_tiles = {}
            e0_chunks = [int(c) for c in plans[0][1]]
            w00a = singles.tile([P, KD * P], F16, tag="w00a")
            nc.sync.dma_start(out=w00a[:], in_=w12_d[:, 0, 0, ds(0, KD * P)])
            w00b = singles.tile([P, KD * P], F16, tag="w00b")
            nc.sync.dma_start(
                out=w00b[:], in_=w12_d[:, 0, 0, ds(KD * P, KD * P)]
            )
            w12_tiles[(0, 0)] = (w00a, w00b)
            for ht in range(1, HTL):
                a = singles.tile([P, KD * P], F16, tag=f"w0{ht}a")
                nc.sync.dma_start(out=a[:], in_=w12_d[:, 0, ht, ds(0, KD * P)])
                b = singles.tile([P, KD * P], F16, tag=f"w0{ht}b")
                nc.sync.dma_start(
                    out=b[:], in_=w12_d[:, 0, ht, ds(KD * P, KD * P)]
                )
                w12_tiles[(0, ht)] = (a, b)

            x_tiles = {}
            x0_tiles = []
            c0_ = 0
            for ci, cw in enumerate(e0_chunks):
                src = x_d[:, ds(KD * (offs[0] + c0_), KD * cw)]
                eng = nc.scalar
                if ci == 0:
                    t = singles.tile([P, KD * cw], F16, tag=f"x0_{ci}")
                    eng.dma_start(out=t[:], in_=src)
                    x0_tiles.append(t)
                else:
                    # k-halved: matmuls k0-3 start on the first half
                    hw_ = (KD // 2) * cw
                    ta = singles.tile([P, hw_], F16, tag=f"x0_{ci}a")
                    eng.dma_start(out=ta[:], in_=src[:, ds(0, hw_)])
                    tb = singles.tile([P, hw_], F16, tag=f"x0_{ci}b")
                    eng.dma_start(out=tb[:], in_=src[:, ds(hw_, hw_)])
                    x0_tiles.append((ta, tb))
                c0_ += cw
            x_tiles[0] = x0_tiles

            # warmup matmuls: keep PE active through the startup head
            wu = singles.tile([P, P], F16, tag="wu")
            nc.vector.memset(wu[:], 0)
            wups = psp.tile([P, 512], F32, tag="ps", name="wu")
            for _ in range(WARMUP):
                nc.tensor.matmul(wups[:, :P], wu[:], wu[:], start=True, stop=True)

            w3_tiles = {}
            hh_tiles = {}

            def prefetch_w12(ei, ht):
                if ei >= E or (ei, ht) in w12_tiles:
                    return
                t = w12p.tile([P, 2 * KD * P], F16, tag="w12")
                nc.scalar.dma_start(out=t[:], in_=w12_d[:, ei, ht, :])
                w12_tiles[(ei, ht)] = t

            def prefetch_x(ei):
                if ei >= E or ei in x_tiles:
                    return
                t = xrp.tile([P, KD * C_max], F16, tag="xres")
                nc.sync.dma_start(
                    out=t[:, ds(0, KD * C_list[ei])],
                    in_=x_d[:, ds(KD * offs[ei], KD * C_list[ei])],
                )
                x_tiles[ei] = t

            def prefetch_w3(ei):
                if ei >= E or ei in w3_tiles:
                    return
                t = w3p.tile([P, HTL * D], F16, tag="w3")
                nc.sync.dma_start(out=t[:], in_=w3_d[:, ei, :])
                w3_tiles[ei] = t

            for ei, (C_e, chunks) in enumerate(plans):
                C_e = int(C_e)
                chunks = [int(c) for c in chunks]
                c_offs = [int(v) for v in np.cumsum([0] + chunks)[:-1]]
                xt = x_tiles[ei]

                def xsrc(ci, c0, k, cw, ei=ei, xt=xt):
                    if ei == 0:
                        t = xt[ci]
                        if isinstance(t, tuple):
                            t = t[0] if k < KD // 2 else t[1]
                            return t[:, ds((k % (KD // 2)) * cw, cw)]
                        return t[:, ds(k * cw, cw)]
                    return xt[:, ds(KD * c0 + k * cw, cw)]

                hh = hhp.tile([P, HTL * C_max], F16, tag="hh")
                hh_tiles[ei] = hh

                # ---- Phase B work-item schedule
                nch = len(chunks)
                if ei == 0 and nch >= 4:
                    # triangular: consume tiles in DMA landing order so
                    # the PE has work from ~10us on (chunks 0/1 + early
                    # h-tiles) while the rest of x streams in
                    items = [(0, 0), (1, 0), (2, 0), (3, 0),
                             (0, 1), (1, 1), (2, 1), (3, 1)]
                    items += [(ht, ci) for ci in range(2, nch)
                              for ht in range(HTL)]
                    hooks = {
                        (2, 0): [lambda: prefetch_x(1)],
                        (3, 0): [lambda: prefetch_w3(0)],
                        (0, 2): [lambda: prefetch_w12(1, 0)],
                        (2, nch - 1): [lambda: prefetch_w12(1, 1)],
                    }
                else:
                    items = [(ht, ci) for ht in range(HTL)
                             for ci in range(nch)]
                    hooks = {}
                    if ei == 0:
                        hooks = {
                            (1, 0): [lambda: prefetch_x(1)],
                            (2, 0): [lambda: prefetch_w3(0)],
                            (3, 0): [lambda: prefetch_w12(1, 0),
                                     lambda: prefetch_w12(1, 1)],
                        }
                seen_ht = set()
                for ht, ci in items:
                    for fn in hooks.get((ht, ci), []):
                        fn()
                    if ei > 0 and ht not in seen_ht:
                        seen_ht.add(ht)
                        if ht + 1 < HTL:
                            prefetch_w12(ei, ht + 1)
                        else:
                            prefetch_w12(ei + 1, 0)
                        if ht == 1:
                            prefetch_x(ei + 1)
                        if ht == 2:
                            prefetch_w3(ei)
                        if ht == 3:
                            prefetch_w12(ei + 1, 1)
                    w12t = w12_tiles[(ei, ht)]
                    if isinstance(w12t, tuple):
                        w1s = lambda k, t=w12t[0]: t[:, ts(k, P)]
                        w2s = lambda k, t=w12t[1]: t[:, ts(k, P)]
                    else:
                        w1s = lambda k, t=w12t: t[:, ts(k, P)]
                        w2s = lambda k, t=w12t: t[:, ds((KD + k) * P, P)]
                    cw, c0 = chunks[ci], c_offs[ci]
                    p1 = psp.tile([P, 512], F32, tag="ps", name="p1")
                    for k in range(KD):
                        nc.tensor.matmul(
                            p1[:, :cw],
                            w1s(k),
                            xsrc(ci, c0, k, cw),
                            start=(k == 0),
                            stop=(k == KD - 1),
                        )
                    p2 = psp.tile([P, 512], F32, tag="ps", name="p2")
                    for k in range(KD):
                        nc.tensor.matmul(
                            p2[:, :cw],
                            w2s(k),
                            xsrc(ci, c0, k, cw),
                            start=(k == 0),
                            stop=(k == KD - 1),
                        )
                    s1 = evp.tile([P, 512], F32, tag="s1")
                    nc.scalar.activation(s1[:, :cw], p1[:, :cw], AF.Silu)
                    nc.vector.tensor_mul(
                        hh[:, ds(ht * C_e + c0, cw)], s1[:, :cw], p2[:, :cw]
                    )

                # ---- Phase C for expert ei: dt outer, kh inner (depth 4)
                w3t = w3_tiles[ei]
                last_e = ei == E - 1
                for ci, (cw, c0) in enumerate(zip(chunks, c_offs)):
                    last_chunk = last_e and ci == len(chunks) - 1
                    for half in range(2):
                        hdt = DT // 2
                        banks = []
                        for i in range(hdt):
                            dt = half * hdt + i
                            bank = psp.tile([P, 512], F32, tag="ps", name=f"pc{i}")
                            banks.append(bank)
                            for kh in range(HTL):
                                nc.tensor.matmul(
                                    bank[:, :cw],
                                    w3t[:, ds(kh * D + dt * P, P)],
                                    hh[:, ds(kh * C_e + c0, cw)],
                                    start=(kh == 0),
                                    stop=(kh == HTL - 1),
                                )
                        if last_chunk and half == 1:
                            # store per-dt so only the final 32KB DMA
                            # trails the last matmul
                            for i in range(hdt):
                                dt = half * hdt + i
                                obS = singles.tile([P, cw], F16, tag=f"obS{i}")
                                eng = (
                                    nc.vector.tensor_copy
                                    if i % 2 == 0
                                    else nc.scalar.copy
                                )
                                eng(obS[:], banks[i][:, :cw])
                                nc.gpsimd.dma_start(
                                    out=out_v[:, dt, ds(offs[ei] + c0, cw)],
                                    in_=obS[:],
                                )
                        else:
                            obL = obp.tile([P, hdt * 512], F16, tag="ob")
                            for i in range(hdt):
                                eng = (
                                    nc.vector.tensor_copy
                                    if i % 2 == 0
                                    else nc.scalar.copy
                                )
                                eng(obL[:, ds(i * cw, cw)], banks[i][:, :cw])
                            nc.gpsimd.dma_start(
                                out=out_v[
                                    :, ds(half * hdt, hdt), ds(offs[ei] + c0, cw)
                                ],
                                in_=obL[:, ds(0, hdt * cw)].rearrange(
                                    "p (t c) -> p t c", t=hdt
                                ),
                            )

            # tail keepalive: dummy matmuls so the HAM clock stays at 8/8
            # while the final evictions/stores drain and the framework
            # teardown (semaphore clears) runs -- otherwise the clock
            # halves ~3.4us after the last real matmul and the teardown
            # itself runs 2x slower.
            wups2 = psp.tile([P, 512], F32, tag="ps", name="wu2")
            for _ in range(80):
                nc.tensor.matmul(wups2[:, :P], wu[:], wu[:], start=True, stop=True)

    nc.compile()
    return nc


_NC_CACHE = {}


def get_nc(plans):
    key = tuple((c, tuple(ch)) for c, ch in plans)
    if key not in _NC_CACHE:
        _NC_CACHE[key] = build_program(plans)
    return _NC_CACHE[key]


def route(x, Wg):
    """Host router: fp32 scores, top-2 of 8, softmax over the pair."""
    s = x @ Wg                                          # [N, E]
    m1 = s.max(-1, keepdims=True)
    masked = np.where(s == m1, -np.inf, s)
    m2 = masked.max(-1, keepdims=True)
    den = 1.0 + np.exp(m2 - m1)
    gates = ((s >= m2) * (np.exp(s - m1) / den)).astype(np.float32)  # [N, E]
    return gates


def prepare(inputs):
    x = np.asarray(inputs["x"], dtype=np.float32).reshape(N, D)
    Wg = np.ascontiguousarray(np.asarray(inputs["Wg"], dtype=np.float32))
    W1 = np.asarray(inputs["W1"], dtype=np.float32)
    W2 = np.asarray(inputs["W2"], dtype=np.float32)
    W3 = np.asarray(inputs["W3"], dtype=np.float32)

    gates = route(x, Wg)
    loads = [int((gates[:, e] > 0).sum()) for e in range(E)]
    order, plans = make_plans(loads)

    idx_list, gate_list = [], []
    xparts = []
    for i, e in enumerate(order):
        C_e, chunks = plans[i]
        idx = np.nonzero(gates[:, e] > 0)[0]
        idx_list.append(idx)
        gate_list.append(gates[idx, e])
        xr = np.zeros((C_e, D), np.float16)
        xr[: len(idx)] = x[idx].astype(np.float16)
        c0 = 0
        for cw in chunks:
            xparts.append(
                xr[c0 : c0 + cw].reshape(cw, KD, P).transpose(2, 1, 0).reshape(P, -1)
            )
            c0 += cw
    x_all = np.ascontiguousarray(np.concatenate(xparts, axis=1))  # [P, KD*C_tot]

    # weight shuffles: [p, ht_global, ...] then slice per core
    # w12_all[p, htg, e, j, k*128+h] = Wj_order[e][k*128+p, htg*128+h]
    HT = H // P
    w1s = W1[order].astype(np.float16)   # [E, D, H]
    w2s = W2[order].astype(np.float16)
    w3s = W3[order].astype(np.float16)   # [E, H, D]
    # [E, KD, P, HT, P] -> [P, HT, E, 2, KD, P]
    w1r = w1s.reshape(E, KD, P, HT, P).transpose(2, 3, 0, 1, 4)
    w2r = w2s.reshape(E, KD, P, HT, P).transpose(2, 3, 0, 1, 4)
    w12_all = np.stack([w1r, w2r], axis=3)  # [P, HT, E, 2, KD, P]
    # w3_all[p, khg, e, d] = W3_order[e][khg*128+p, d]
    w3_all = w3s.reshape(E, HT, P, D).transpose(2, 1, 0, 3)  # [P, HT, E, D]

    in_maps = []
    for core in range(NCORES):
        hsl = slice(core * HTL, (core + 1) * HTL)
        w12c = np.ascontiguousarray(
            w12_all[:, hsl].transpose(0, 2, 1, 3, 4, 5).reshape(P, E, HTL, -1)
        )
        w3c = np.ascontiguousarray(
            w3_all[:, hsl].transpose(0, 2, 1, 3).reshape(P, E, HTL * D)
        )
        in_maps.append({"xc": x_all, "W12": w12c, "W3e": w3c})
    return in_maps, plans, order, idx_list, gate_list


def combine(res, plans, idx_list, gate_list):
    """Sum the 8 per-core h-slice partials, gate, scatter-add."""
    psum = res.results[0]["out"].astype(np.float32)
    for core in range(1, NCORES):
        psum += res.results[core]["out"]
    out = np.zeros((N, D), np.float32)
    off = 0
    for i, (C_e, _) in enumerate(plans):
        idx = idx_list[i]
        L = len(idx)
        out[idx] += psum[:, off : off + L].T * gate_list[i][:, None]
        off += C_e
    return out.reshape(B, S, D)


def run_spmd(nc, in_maps, trace=False, **kw):
    return run_bass_kernel_spmd(
        nc, in_maps, core_ids=list(range(NCORES)), trace=trace, **kw
    )


def kernel(**inputs):
    in_maps, plans, order, idx_list, gate_list = prepare(inputs)
    nc = get_nc(plans)
    res = run_spmd(nc, in_maps)
    return combine(res, plans, idx_list, gate_list)


# revision 42
# speedup vs baseline: 1.0063x; 1.0063x over previous
"""MoE FeedForward (top-2 of 8 experts, SwiGLU) for 8 Trainium2 NeuronCores.

H-sharded layout (v4): instead of expert-parallel (one expert per core,
capacity = max expert load = 1092), every core processes ALL routed
(token, expert) pairs but only a 512-wide slice of the hidden dim H
(h-tiles 4i..4i+3 for core i).  The MoE combine (host) sums the 8
partial outputs and applies the gates.  Per-core PE work is exactly
sum(L_e)/8-equivalent = 1024 token-slots -- perfect balance, ~6% less
matmul streaming than the expert-parallel v3.

The host router (fp32 top-2 + softmax) runs before build_program, so
the per-expert token counts C_e are compile-time constants: the program
loops over experts with exact trip counts.  All cores run the SAME
program (SPMD); they differ only in which h-slice of W1/W2/W3 their
input tensors contain.

Per-core phases, per expert e (in process order):
  B(e): hh[ht, tok] = silu(W1_e[:, hsl].T @ x_e) * (W2_e[:, hsl].T @ x_e)
        for the core's 4 h-tiles; ht outer, token-chunks inner; x_e
        resident (2.2 MB), weights streamed once (524 KB/h-tile).
  C(e): out_partial[d, tok] = W3_e[hsl, :].T @ hh  -- accumulation
        depth only 4 k-tiles; dt outer / kh inner so each PSUM bank
        finishes fast and evictions overlap the next bank's matmuls.
C(e) is emitted after B(e); B(e+1) follows, its DMAs prefetched during
B(e), so the PE stream never waits on memory in steady state.

All matmuls fp16 (PE full rate; fp8 would need DoubleRow but its
~2.6% quantization rms blows the 2e-2 rel-err budget -- measured).
"""

import contextlib

import numpy as np

import concourse.bacc as bacc
import concourse.tile as tile
from concourse import mybir
from concourse.bass import ds, ts
from concourse.bass_utils import run_bass_kernel_spmd

AF = mybir.ActivationFunctionType
F32 = mybir.dt.float32
F16 = mybir.dt.float16

# Problem shape (hardcoded per contract)
B, S, D, H, E = 2, 2048, 1024, 4096, 8
N = B * S            # 4096 tokens
TOP_K = 2
NCORES = 8

P = 128              # SBUF partitions
KD = D // P          # 8 k-tiles over D
HTL = (H // P) // NCORES  # 4 h-tiles (of 128) per core
DT = D // P          # 8 d-tiles (phase C stationary tiles)
WARMUP = 24          # dummy matmuls to hold the HAM clock at 8/8
                     # through the ~8.5us startup head


def round8(v):
    return (v + 7) & ~7


def round4(v):
    return (v + 3) & ~3


def make_plans(loads):
    """Per-expert (C_e, chunk list) in process order, plus the order.

    Experts are processed largest-load first (the last expert is the
    smallest so the kernel tail is short).  The first expert leads with
    progressively growing chunks (128/256/360) so early matmuls depend
    on small DMAs; the last expert trails with a 64-token chunk so the
    final eviction+store tail after the last matmul is tiny.
    """
    order = sorted(range(E), key=lambda e: -loads[e])
    plans = []
    for i, e in enumerate(order):
        C_e = round4(max(loads[e], 8))
        rem = C_e
        lead, tail = [], []
        if i == 0 and C_e >= 752:
            lead = [128, 256, 360]
            rem -= 744
        if i == E - 1 and rem >= 72:
            tail = [64]
            rem -= 64
        n = max(1, -(-rem // 512))
        base = round8(-(-rem // n))
        mids = []
        while rem > 0:
            c = min(base, rem)
            mids.append(c)
            rem -= c
        assert all(c % 4 == 0 and 0 < c <= 512 for c in lead + mids + tail)
        plans.append((C_e, lead + mids + tail))
    return order, plans


def build_program(plans):
    nc = bacc.Bacc(
        "TRN2",
        target_bir_lowering=False,
        debug=False,
        enable_asserts=False,
        num_devices=NCORES,
    )
    C_list = [int(p[0]) for p in plans]
    C_tot = sum(C_list)
    C_max = max(C_list)
    offs = [int(v) for v in np.cumsum([0] + C_list)[:-1]]

    # Host-pre-shuffled layouts (see make_in_maps):
    #   x_d [p, KD*off_e + KD*c0 + k*cw + t] = x_routed_e[c0+t, k*128+p]
    #     (chunk-major within each expert block)
    #   w12_d[p, e, ht, j*KD*128 + k*128 + h] = Wj_e[k*128+p, (4i+ht)*128+h]
    #   w3_d [p, e, kh*D + d]                 = W3_e[(4i+kh)*128+p, d]
    #   out_d[D, C_tot] f16 partials over this core's h-slice (fp16
#     store halves the output traffic; the host combine sums in f32 and
#     the rounding is ~1e-5 of the 2e-2 budget)
    x_d = nc.dram_tensor("xc", [P, KD * C_tot], F16, kind="ExternalInput").ap()
    w12_d = nc.dram_tensor(
        "W12", [P, E, HTL, 2 * KD * P], F16, kind="ExternalInput"
    ).ap()
    w3_d = nc.dram_tensor("W3e", [P, E, HTL * D], F16, kind="ExternalInput").ap()
    out_d = nc.dram_tensor("out", [D, C_tot], F16, kind="ExternalOutput").ap()
    out_v = out_d.rearrange("(dt p) c -> p dt c", p=P)    # [128, DT, C_tot]

    with tile.TileContext(nc) as tc:
        with contextlib.ExitStack() as ctx:
            singles = ctx.enter_context(tc.tile_pool(name="singles", bufs=1))
            w12p = ctx.enter_context(tc.tile_pool(name="w12", bufs=6))
            xrp = ctx.enter_context(tc.tile_pool(name="xr", bufs=2))
            hhp = ctx.enter_context(tc.tile_pool(name="hh", bufs=2))
            w3p = ctx.enter_context(tc.tile_pool(name="w3", bufs=2))
            evp = ctx.enter_context(tc.tile_pool(name="ev", bufs=3))
            obp = ctx.enter_context(tc.tile_pool(name="ob", bufs=6))
            psp = ctx.enter_context(tc.tile_pool(name="ps", bufs=8, space="PSUM"))

            # --- startup DMAs -------------------------------------------------
            # Early per-queue DMA bandwidth is low (~100-200 GB/s while
            # the queues ramp from ~8.5us), so the first-matmul
            # dependencies are spread over the three DMA rings and split
            # into small pieces, each its own tile (tile-granular
            # dependency tracking would otherwise make the first matmul
            # wait for all of them):
            #   sync:   w12(e0,ht0) as W1|W2 halves, then w12(e0,ht1..3)
            #   scalar: x0 chunks 0,2,3 (later: w12 streaming)
            #   gpsimd: x0 chunk 1    (later: output stores)
            # Expert 0's phase B runs a triangular (ht, chunk) schedule
            # that consumes tiles in roughly this landing order.
            w12_tiles = {}
            e0_chunks = [int(c) for c in plans[0][1]]
            w00a = singles.tile([P, KD * P], F16, tag="w00a")
            nc.sync.dma_start(out=w00a[:], in_=w12_d[:, 0, 0, ds(0, KD * P)])
            w00b = singles.tile([P, KD * P], F16, tag="w00b")
            nc.sync.dma_start(
                out=w00b[:], in_=w12_d[:, 0, 0, ds(KD * P, KD * P)]
            )
            w12_tiles[(0, 0)] = (w00a, w00b)
            for ht in range(1, HTL):
                a = singles.tile([P, KD * P], F16, tag=f"w0{ht}a")
                nc.sync.dma_start(out=a[:], in_=w12_d[:, 0, ht, ds(0, KD * P)])
                b = singles.tile([P, KD * P], F16, tag=f"w0{ht}b")
                nc.sync.dma_start(
                    out=b[:], in_=w12_d[:, 0, ht, ds(KD * P, KD * P)]
                )
                w12_tiles[(0, ht)] = (a, b)

            x_tiles = {}
            x0_tiles = []
            c0_ = 0
            for ci, cw in enumerate(e0_chunks):
                src = x_d[:, ds(KD * (offs[0] + c0_), KD * cw)]
                eng = nc.gpsimd if ci == 1 else nc.scalar
                if ci == 0:
                    t = singles.tile([P, KD * cw], F16, tag=f"x0_{ci}")
                    eng.dma_start(out=t[:], in_=src)
                    x0_tiles.append(t)
                else:
                    # k-halved: matmuls k0-3 start on the first half
                    hw_ = (KD // 2) * cw
                    ta = singles.tile([P, hw_], F16, tag=f"x0_{ci}a")
                    eng.dma_start(out=ta[:], in_=src[:, ds(0, hw_)])
                    tb = singles.tile([P, hw_], F16, tag=f"x0_{ci}b")
                    eng.dma_start(out=tb[:], in_=src[:, ds(hw_, hw_)])
                    x0_tiles.append((ta, tb))
                c0_ += cw
            x_tiles[0] = x0_tiles

            # warmup matmuls: keep PE active through the startup head
            wu = singles.tile([P, P], F16, tag="wu")
            nc.vector.memset(wu[:], 0)
            wups = psp.tile([P, 512], F32, tag="ps", name="wu")
            for _ in range(WARMUP):
                nc.tensor.matmul(wups[:, :P], wu[:], wu[:], start=True, stop=True)

            w3_tiles = {}
            hh_tiles = {}

            def prefetch_w12(ei, ht):
                if ei >= E or (ei, ht) in w12_tiles:
                    return
                t = w12p.tile([P, 2 * KD * P], F16, tag="w12")
                nc.scalar.dma_start(out=t[:], in_=w12_d[:, ei, ht, :])
                w12_tiles[(ei, ht)] = t

            def prefetch_x(ei):
                if ei >= E or ei in x_tiles:
                    return
                t = xrp.tile([P, KD * C_max], F16, tag="xres")
                nc.sync.dma_start(
                    out=t[:, ds(0, KD * C_list[ei])],
                    in_=x_d[:, ds(KD * offs[ei], KD * C_list[ei])],
                )
                x_tiles[ei] = t

            def prefetch_w3(ei):
                if ei >= E or ei in w3_tiles:
                    return
                t = w3p.tile([P, HTL * D], F16, tag="w3")
                nc.sync.dma_start(out=t[:], in_=w3_d[:, ei, :])
                w3_tiles[ei] = t

            for ei, (C_e, chunks) in enumerate(plans):
                C_e = int(C_e)
                chunks = [int(c) for c in chunks]
                c_offs = [int(v) for v in np.cumsum([0] + chunks)[:-1]]
                xt = x_tiles[ei]

                def xsrc(ci, c0, k, cw, ei=ei, xt=xt):
                    if ei == 0:
                        t = xt[ci]
                        if isinstance(t, tuple):
                            t = t[0] if k < KD // 2 else t[1]
                            return t[:, ds((k % (KD // 2)) * cw, cw)]
                        return t[:, ds(k * cw, cw)]
                    return xt[:, ds(KD * c0 + k * cw, cw)]

                hh = hhp.tile([P, HTL * C_max], F16, tag="hh")
                hh_tiles[ei] = hh

                # ---- Phase B work-item schedule
                nch = len(chunks)
                if ei == 0 and nch >= 4:
                    # triangular: consume tiles in DMA landing order so
                    # the PE has work from ~10us on (chunks 0/1 + early
                    # h-tiles) while the rest of x streams in
                    items = [(0, 0), (1, 0), (2, 0), (3, 0),
                             (0, 1), (1, 1), (2, 1), (3, 1)]
                    items += [(ht, ci) for ci in range(2, nch)
                              for ht in range(HTL)]
                    hooks = {
                        (2, 0): [lambda: prefetch_x(1)],
                        (3, 0): [lambda: prefetch_w3(0)],
                        (0, 2): [lambda: prefetch_w12(1, 0)],
                        (2, nch - 1): [lambda: prefetch_w12(1, 1)],
                    }
                else:
                    items = [(ht, ci) for ht in range(HTL)
                             for ci in range(nch)]
                    hooks = {}
                    if ei == 0:
                        hooks = {
                            (1, 0): [lambda: prefetch_x(1)],
                            (2, 0): [lambda: prefetch_w3(0)],
                            (3, 0): [lambda: prefetch_w12(1, 0),
                                     lambda: prefetch_w12(1, 1)],
                        }
                seen_ht = set()
                for ht, ci in items:
                    for fn in hooks.get((ht, ci), []):
                        fn()
                    if ei > 0 and ht not in seen_ht:
                        seen_ht.add(ht)
                        if ht + 1 < HTL:
                            prefetch_w12(ei, ht + 1)
                        else:
                            prefetch_w12(ei + 1, 0)
                        if ht == 1:
                            prefetch_x(ei + 1)
                        if ht == 2:
                            prefetch_w3(ei)
                        if ht == 3:
                            prefetch_w12(ei + 1, 1)
                    w12t = w12_tiles[(ei, ht)]
                    if isinstance(w12t, tuple):
                        w1s = lambda k, t=w12t[0]: t[:, ts(k, P)]
                        w2s = lambda k, t=w12t[1]: t[:, ts(k, P)]
                    else:
                        w1s = lambda k, t=w12t: t[:, ts(k, P)]
                        w2s = lambda k, t=w12t: t[:, ds((KD + k) * P, P)]
                    cw, c0 = chunks[ci], c_offs[ci]
                    p1 = psp.tile([P, 512], F32, tag="ps", name="p1")
                    for k in range(KD):
                        nc.tensor.matmul(
                            p1[:, :cw],
                            w1s(k),
                            xsrc(ci, c0, k, cw),
                            start=(k == 0),
                            stop=(k == KD - 1),
                        )
                    p2 = psp.tile([P, 512], F32, tag="ps", name="p2")
                    for k in range(KD):
                        nc.tensor.matmul(
                            p2[:, :cw],
                            w2s(k),
                            xsrc(ci, c0, k, cw),
                            start=(k == 0),
                            stop=(k == KD - 1),
                        )
                    s1 = evp.tile([P, 512], F32, tag="s1")
                    nc.scalar.activation(s1[:, :cw], p1[:, :cw], AF.Silu)
                    nc.vector.tensor_mul(
                        hh[:, ds(ht * C_e + c0, cw)], s1[:, :cw], p2[:, :cw]
                    )

                # ---- Phase C for expert ei: dt outer, kh inner (depth 4)
                w3t = w3_tiles[ei]
                last_e = ei == E - 1
                for ci, (cw, c0) in enumerate(zip(chunks, c_offs)):
                    last_chunk = last_e and ci == len(chunks) - 1
                    for half in range(2):
                        hdt = DT // 2
                        banks = []
                        for i in range(hdt):
                            dt = half * hdt + i
                            bank = psp.tile([P, 512], F32, tag="ps", name=f"pc{i}")
                            banks.append(bank)
                            for kh in range(HTL):
                                nc.tensor.matmul(
                                    bank[:, :cw],
                                    w3t[:, ds(kh * D + dt * P, P)],
                                    hh[:, ds(kh * C_e + c0, cw)],
                                    start=(kh == 0),
                                    stop=(kh == HTL - 1),
                                )
                        if last_chunk and half == 1:
                            # store per-dt so only the final 32KB DMA
                            # trails the last matmul
                            for i in range(hdt):
                                dt = half * hdt + i
                                obS = singles.tile([P, cw], F16, tag=f"obS{i}")
                                eng = (
                                    nc.vector.tensor_copy
                                    if i % 2 == 0
                                    else nc.scalar.copy
                                )
                                eng(obS[:], banks[i][:, :cw])
                                nc.gpsimd.dma_start(
                                    out=out_v[:, dt, ds(offs[ei] + c0, cw)],
                                    in_=obS[:],
                                )
                        else:
                            obL = obp.tile([P, hdt * 512], F16, tag="ob")
                            for i in range(hdt):
                                eng = (
                                    nc.vector.tensor_copy
                                    if i % 2 == 0
                                    else nc.scalar.copy
                                )
                                eng(obL[:, ds(i * cw, cw)], banks[i][:, :cw])
                            nc.gpsimd.dma_start(
                                out=out_v[
                                    :, ds(half * hdt, hdt), ds(offs[ei] + c0, cw)
                                ],
                                in_=obL[:, ds(0, hdt * cw)].rearrange(
                                    "p (t c) -> p t c", t=hdt
                                ),
                            )

            # tail keepalive: dummy matmuls so the HAM clock stays at 8/8
            # while the final evictions/stores drain and the framework
            # teardown (semaphore clears) runs -- otherwise the clock
            # halves ~3.4us after the last real matmul and the teardown
            # itself runs 2x slower.
            wups2 = psp.tile([P, 512], F32, tag="ps", name="wu2")
            for _ in range(80):
                nc.tensor.matmul(wups2[:, :P], wu[:], wu[:], start=True, stop=True)

    nc.compile()
    return nc


_NC_CACHE = {}


def get_nc(plans):
    key = tuple((c, tuple(ch)) for c, ch in plans)
    if key not in _NC_CACHE:
        _NC_CACHE[key] = build_program(plans)
    return _NC_CACHE[key]


def route(x, Wg):
    """Host router: fp32 scores, top-2 of 8, softmax over the pair."""
    s = x @ Wg                                          # [N, E]
    m1 = s.max(-1, keepdims=True)
    masked = np.where(s == m1, -np.inf, s)
    m2 = masked.max(-1, keepdims=True)
    den = 1.0 + np.exp(m2 - m1)
    gates = ((s >= m2) * (np.exp(s - m1) / den)).astype(np.float32)  # [N, E]
    return gates


def prepare(inputs):
    x = np.asarray(inputs["x"], dtype=np.float32).reshape(N, D)
    Wg = np.ascontiguousarray(np.asarray(inputs["Wg"], dtype=np.float32))
    W1 = np.asarray(inputs["W1"], dtype=np.float32)
    W2 = np.asarray(inputs["W2"], dtype=np.float32)
    W3 = np.asarray(inputs["W3"], dtype=np.float32)

    gates = route(x, Wg)
    loads = [int((gates[:, e] > 0).sum()) for e in range(E)]
    order, plans = make_plans(loads)

    idx_list, gate_list = [], []
    xparts = []
    for i, e in enumerate(order):
        C_e, chunks = plans[i]
        idx = np.nonzero(gates[:, e] > 0)[0]
        idx_list.append(idx)
        gate_list.append(gates[idx, e])
        xr = np.zeros((C_e, D), np.float16)
        xr[: len(idx)] = x[idx].astype(np.float16)
        c0 = 0
        for cw in chunks:
            xparts.append(
                xr[c0 : c0 + cw].reshape(cw, KD, P).transpose(2, 1, 0).reshape(P, -1)
            )
            c0 += cw
    x_all = np.ascontiguousarray(np.concatenate(xparts, axis=1))  # [P, KD*C_tot]

    # weight shuffles: [p, ht_global, ...] then slice per core
    # w12_all[p, htg, e, j, k*128+h] = Wj_order[e][k*128+p, htg*128+h]
    HT = H // P
    w1s = W1[order].astype(np.float16)   # [E, D, H]
    w2s = W2[order].astype(np.float16)
    w3s = W3[order].astype(np.float16)   # [E, H, D]
    # [E, KD, P, HT, P] -> [P, HT, E, 2, KD, P]
    w1r = w1s.reshape(E, KD, P, HT, P).transpose(2, 3, 0, 1, 4)
    w2r = w2s.reshape(E, KD, P, HT, P).transpose(2, 3, 0, 1, 4)
    w12_all = np.stack([w1r, w2r], axis=3)  # [P, HT, E, 2, KD, P]
    # w3_all[p, khg, e, d] = W3_order[e][khg*128+p, d]
    w3_all = w3s.reshape(E, HT, P, D).transpose(2, 1, 0, 3)  # [P, HT, E, D]

    in_maps = []
    for core in range(NCORES):
        hsl = slice(core * HTL, (core + 1) * HTL)
        w12c = np.ascontiguousarray(
            w12_all[:, hsl].transpose(0, 2, 1, 3, 4, 5).reshape(P, E, HTL, -1)
        )
        w3c = np.ascontiguousarray(
            w3_all[:, hsl].transpose(0, 2, 1, 3).reshape(P, E, HTL * D)
        )
        in_maps.append({"xc": x_all, "W12": w12c, "W3e": w3c})
    return in_maps, plans, order, idx_list, gate_list


def combine(res, plans, idx_list, gate_list):
    """Sum the 8 per-core h-slice partials, gate, scatter-add."""
    psum = res.results[0]["out"].astype(np.float32)
    for core in range(1, NCORES):
        psum += res.results[core]["out"]
    out = np.zeros((N, D), np.float32)
    off = 0
    for i, (C_e, _) in enumerate(plans):
        idx = idx_list[i]
        L = len(idx)
        out[idx] += psum[:, off : off + L].T * gate_list[i][:, None]
        off += C_e
    return out.reshape(B, S, D)


def run_spmd(nc, in_maps, trace=False, **kw):
    return run_bass_kernel_spmd(
        nc, in_maps, core_ids=list(range(NCORES)), trace=trace, **kw
    )


def kernel(**inputs):
    in_maps, plans, order, idx_list, gate_list = prepare(inputs)
    nc = get_nc(plans)
    res = run_spmd(nc, in_maps)
    return combine(res, plans, idx_list, gate_list)
